# revision 12
# baseline (speedup 1.0000x reference)
"""TRN2 Bass kernel for nn_Attention_11252814315826.

out[b,h,s,:] = softmax(Q[b,h] @ K^T[b,h] / 8 + addr(mask)) @ V[b,h]
with the additive mask on the QUERY dim: for mask[b,s]==0 the reference's
-1e12 row offset makes softmax exactly uniform, so out = colmean(V[b,h]).

Strategy (v2): shard the 32 (b,h) pairs 4-per-core across 8 NeuronCores.
Host-side, compact query rows to the mask==1 subset, transpose to qT
[64, SP] bf16 (SP = max_cnt+1; one zero column supplies colmean(V) for
all masked rows), and duplicate qT/kT onto both PE row halves so QK
matmuls run row-tiled: two key-blocks concurrently on PE rows 0-63 /
64-127 (K=64 each) for ~2x QK throughput.

Scores for 3 key-blocks accumulate in one [128, 3*512] PSUM tile so a
single ACTIVATE(Exp) covers N=1536 (amortizes the ~310-cycle ScalarE
per-instruction overhead; exp is the roofline engine at 1 elem/cycle/
lane @ 1.2 GHz). PV accumulates [65, sw] (V plus a ones column for the
softmax denominator) in PSUM over all 16 key-blocks; epilogue is a DVE
copy PSUM->SBUF and a DMA of the raw [65, SP] (d-major) result. The
host divides by the denominator row and transposes/scatters back.
"""

import os
import sys

for _p in (
    "/root/.axon_site",
    "/root/.axon_site/_ro/trn_rl_repo",
    "/root/.axon_site/_ro/pypackages",
    "/opt/trn_rl_repo",
):
    if os.path.isdir(_p) and _p not in sys.path:
        sys.path.append(_p)

from concourse.bass_utils import run_bass_kernel_spmd

import numpy as np

import concourse.bacc as bacc
import concourse.tile as tile
import concourse.mybir as mybir

F32 = mybir.dt.float32
BF16 = mybir.dt.bfloat16

B, H = 2, 16
S, D = 2048, 64
N_CORES = 8
PAIRS_PER_CORE = (B * H) // N_CORES  # 4
CH = 512  # max chunk width (psum bank = 512 f32)


def _chunks(total, size):
    out, s0 = [], 0
    while s0 < total:
        w = min(size, total - s0)
        out.append((s0, w))
        s0 += w
    return out


def _groups(sw, nt):
    """Split nt key-blocks into ACT groups. Full chunks use 3-block groups
    (N=1536 exp per ACTIVATE, 3 psum banks); narrow chunks pack up to
    8 blocks while each matmul output stays inside one 2KB psum bank."""
    if sw == CH:
        sizes = []
        left = nt
        while left > 0:
            g = min(3, left)
            sizes.append(g)
            left -= g
    else:
        # keep every [128, sw] f32 matmul output within one bank
        per = max(1, min(nt, 2048 // (sw * 4)))
        sizes = []
        left = nt
        while left > 0:
            g = min(per, left)
            sizes.append(g)
            left -= g
    out, t0 = [], 0
    for g in sizes:
        out.append((t0, g))
        t0 += g
    return out


def build_attention_nc(NP=4, SP=1047, S_=2048, D_=64, row_tile=True, repeat=1):
    assert D_ == 64
    NT = S_ // 128

    nc = bacc.Bacc("TRN2", target_bir_lowering=False, debug=False)

    qt = nc.dram_tensor("qt", [NP, D_, SP], BF16, kind="ExternalInput")
    kt = nc.dram_tensor("kt", [NP, D_, S_], BF16, kind="ExternalInput")
    v = nc.dram_tensor("v", [NP, S_, D_], BF16, kind="ExternalInput")
    o = nc.dram_tensor("o", [NP, D_ + 1, SP], F32, kind="ExternalOutput")

    import ml_dtypes

    ones_dram = nc.inline_tensor(
        np.ones((128, NT, 1), dtype=ml_dtypes.bfloat16), name="onescol"
    )
    zeros_dram = nc.inline_tensor(
        np.zeros((64, 640), dtype=ml_dtypes.bfloat16), name="wuzeros"
    )
    dma = nc.sync

    chunks = _chunks(SP, CH)

    with tile.TileContext(nc) as tc:
        with (
            tc.tile_pool(name="kt", bufs=2) as kt_pool,
            tc.tile_pool(name="v", bufs=2) as v_pool,
            tc.tile_pool(name="qt", bufs=2) as qt_pool,
            tc.tile_pool(name="exp", bufs=3) as exp_pool,
            tc.tile_pool(name="osb", bufs=2) as osb_pool,
            tc.tile_pool(name="qkps", bufs=2, space="PSUM") as qk_psum,
            tc.tile_pool(name="pvps", bufs=2, space="PSUM") as pv_psum,
        ):
            ctxs = {}

            def warmup():
                # HAM unthrottle needs one fully-busy ~3.4us PE window; the
                # steady state has micro-gaps that prevent warm-up (but once
                # warm, micro-gaps never re-throttle). Run a dense dummy-MM
                # burst at start, overlapped with the prologue DMAs.
                wz = exp_pool.tile([64, 640], BF16, tag="wz", bufs=1)
                dma.dma_start(wz[:], zeros_dram.ap())
                wt = qk_psum.tile([128, 3 * CH], F32, tag="qk")
                for j in range(18):
                    nc.tensor.matmul(
                        wt[:, (j % 3) * CH : (j % 3) * CH + CH],
                        wz[:, 0:128],
                        wz[:, 128:640],
                        start=True,
                        stop=True,
                    )

            def pair_prologue(p):
                # kT duplicated on both partition halves for row-tiling
                kt_sb = kt_pool.tile([128, S_], BF16, tag="kt")
                for h in range(2):
                    for c0 in range(0, S_, S_ // 2):
                        dma.dma_start(
                            kt_sb[h * 64 : h * 64 + 64, c0 : c0 + S_ // 2],
                            kt.ap()[p][:, c0 : c0 + S_ // 2],
                        )
                v_sb = v_pool.tile([128, NT, D_ + 1], BF16, tag="v")
                v_src = v.ap()[p].rearrange("(t p) d -> p t d", p=128)
                for t0 in range(0, NT, NT // 2):
                    dma.dma_start(
                        v_sb[:, t0 : t0 + NT // 2, 0:D_],
                        v_src[:, t0 : t0 + NT // 2, :],
                    )
                dma.dma_start(v_sb[:, :, D_ : D_ + 1], ones_dram.ap())
                # qT duplicated on both halves
                qt_sb = qt_pool.tile([128, SP], BF16, tag="qt")
                for h in range(2):
                    dma.dma_start(qt_sb[h * 64 : h * 64 + 64, :], qt.ap()[p])
                ctxs[p] = dict(kt=kt_sb, v=v_sb, qt=qt_sb)

            def emit_qk_group(p, s0, sw, t0g, glen):
                """QK matmuls for key-blocks t0g..t0g+glen-1, row-tiled by
                parity (half 0 / half 1 run concurrently on the PE)."""
                cx = ctxs[p]
                kt_sb, qt_sb = cx["kt"], cx["qt"]
                qk_t = qk_psum.tile([128, 3 * CH], F32, tag="qk")
                for j in range(glen):
                    t = t0g + j
                    h = (t % 2) if row_tile else 0
                    nc.tensor.matmul(
                        qk_t[:, j * sw : (j + 1) * sw],
                        kt_sb[h * 64 : h * 64 + 64, t * 128 : (t + 1) * 128],
                        qt_sb[h * 64 : h * 64 + 64, s0 : s0 + sw],
                        start=True,
                        stop=True,
                        tile_position=(h * 64, 0) if row_tile else None,
                    )
                return qk_t

            def emit_exp(p, sw, glen, qk_t):
                exp_t = exp_pool.tile([128, 3 * CH], BF16, tag="exp")
                nc.scalar.activation(
                    exp_t[:, 0 : glen * sw],
                    qk_t[:, 0 : glen * sw],
                    mybir.ActivationFunctionType.Exp,
                    scale=0.125,
                )
                return exp_t

            def make_pv(p, sw, t0g, glen, exp_t, pv_t):
                def emit():
                    v_sb = ctxs[p]["v"]
                    for j in range(glen):
                        t = t0g + j
                        nc.tensor.matmul(
                            pv_t[:, 0:sw],
                            v_sb[:, t, :],
                            exp_t[:, j * sw : (j + 1) * sw],
                            start=(t == 0),
                            stop=(t == NT - 1),
                            skip_group_check=True,
                        )

                return emit

            def make_epilogue(p, s0, sw, pv_t):
                def emit():
                    o_sb = osb_pool.tile([D_ + 1, CH], F32, tag="osb")
                    nc.vector.tensor_copy(o_sb[:, 0:sw], pv_t[:, 0:sw])
                    dma.dma_start(o.ap()[p][:, s0 : s0 + sw], o_sb[:, 0:sw])

                return emit

            # flat software-pipelined emission: PE stream is
            # [... QK(g), PV(g-2), QK(g+1), PV(g-1) ...] so the scalar
            # engine's exp(g) hides under PE work and never gates QK.
            def emit_body():
                gstep = [0]
                pvq = []
                epiq = []

                def tick():
                    gstep[0] += 1
                    while epiq and epiq[0][0] <= gstep[0]:
                        epiq.pop(0)[1]()
                    if len(pvq) >= 2:
                        pvq.pop(0)()

                warmup()
                for p in range(NP):
                    pair_prologue(p)
                    for s0, sw in chunks:
                        pv_t = pv_psum.tile([D_ + 1, CH], F32, tag="pv")
                        for t0g, glen in _groups(sw, NT):
                            qk_t = emit_qk_group(p, s0, sw, t0g, glen)
                            exp_t = emit_exp(p, sw, glen, qk_t)
                            tick()
                            pvq.append(make_pv(p, sw, t0g, glen, exp_t, pv_t))
                        epiq.append((gstep[0] + 3, make_epilogue(p, s0, sw, pv_t)))
                while pvq:
                    pvq.pop(0)()
                for _, fn in epiq:
                    fn()

            if repeat == 1:
                emit_body()
            else:
                with tc.For_i(0, repeat, 1):
                    emit_body()

    nc.compile()
    return nc


_NC_CACHE = {}
last_results = None


def _install_profile_hook():
    """Wire up the axon NTFF profiling hook if the image's antenv lacks it."""
    import types

    try:
        import antenv.axon_hooks  # noqa: F401

        return
    except ImportError:
        pass
    try:
        from trn_agent_boot.trn_boot import _ntff_profile_via_ctypes

        hook = _ntff_profile_via_ctypes("/opt/axon/libaxon_pjrt.so")
    except Exception:
        hook = None
    mod = types.ModuleType("antenv.axon_hooks")
    mod._hook = hook
    mod.get_axon_ntff_profile_hook = lambda: mod._hook
    mod.set_axon_ntff_profile_hook = lambda h: setattr(mod, "_hook", h)
    sys.modules["antenv.axon_hooks"] = mod
    import antenv

    antenv.axon_hooks = mod
    import concourse.bass_utils as _bu

    _bu.upload_artifacts = lambda tmpdir: "local://" + tmpdir


def kernel(query, key, value, mask):
    """Full-input attention; shards over 8 NeuronCores internally."""
    global last_results
    import ml_dtypes

    query = np.asarray(query)
    key = np.asarray(key)
    value = np.asarray(value)
    mask = np.asarray(mask)

    idx = [np.nonzero(mask[b] != 0)[0] for b in range(B)]
    cnt = [len(ix) for ix in idx]
    # one zero-padded qT column per batch supplies colmean(V) for masked rows;
    # keep SP even so bf16 DMA rows stay 4-byte aligned
    SP = max(cnt) + (1 if min(cnt) < S else 0)
    SP += SP % 2

    nc = _NC_CACHE.get(SP)
    if nc is None:
        nc = _NC_CACHE[SP] = build_attention_nc(
            NP=PAIRS_PER_CORE,
            SP=SP,
            row_tile=os.environ.get("KERNEL_ROW_TILE", "1") == "1",
        )

    in_maps = []
    for c in range(N_CORES):
        qs = np.zeros((PAIRS_PER_CORE, D, SP), dtype=ml_dtypes.bfloat16)
        ks = np.empty((PAIRS_PER_CORE, D, S), dtype=ml_dtypes.bfloat16)
        vs = np.empty((PAIRS_PER_CORE, S, D), dtype=ml_dtypes.bfloat16)
        for i in range(PAIRS_PER_CORE):
            pair = c * PAIRS_PER_CORE + i
            b, h = pair // H, pair % H
            qs[i, :, : cnt[b]] = query[b, h, idx[b]].T
            ks[i] = key[b, h]
            vs[i] = value[b, h]
        in_maps.append({"qt": qs, "kt": ks, "v": vs})

    trace = os.environ.get("KERNEL_PROFILE", "") == "1"
    if trace:
        _install_profile_hook()
        try:
            import jax

            jax.device_put(
                np.zeros((4,), np.float32), jax.devices()[0]
            ).block_until_ready()
        except Exception as e:
            print(f"profile warmup failed ({e}); disabling trace", file=sys.stderr)
            trace = False
    res = run_bass_kernel_spmd(nc, in_maps, core_ids=list(range(N_CORES)), trace=trace)
    last_results = res

    out = np.empty((B, H, S, D), dtype=np.float32)
    for c in range(N_CORES):
        oc = res.results[c]["o"]  # [NP, D+1, SP] f32 (raw PV + denominator row)
        for i in range(PAIRS_PER_CORE):
            pair = c * PAIRS_PER_CORE + i
            b, h = pair // H, pair % H
            on = oc[i, :D, :] / oc[i, D : D + 1, :]
            out[b, h, idx[b]] = on[:, : cnt[b]].T
            if cnt[b] < S:
                out[b, h, np.nonzero(mask[b] == 0)[0]] = on[:, cnt[b]]
    return out


# revision 20
# speedup vs baseline: 1.3356x; 1.3356x over previous
"""TRN2 Bass kernel for nn_Attention_11252814315826.

out[b,h,s,:] = softmax(Q[b,h] @ K^T[b,h] / 8 + addr(mask)) @ V[b,h]
with the additive mask on the QUERY dim: for mask[b,s]==0 the reference's
-1e12 row offset makes softmax exactly uniform, so out = colmean(V[b,h]).

Strategy: shard the 32 (b,h) pairs 4-per-core across 8 NeuronCores.
Host-side, compact query rows to the mask==1 subset and transpose to qT
[64, SP] bf16 (SP = max_cnt+pad; one zero qT column yields uniform
attention on device, supplying colmean(V) for all masked rows). V gets
a host-appended ones column so the PV matmul also accumulates the
softmax denominator.

Per pair, per 512-wide query chunk: QK matmuls write scores for 3
key-blocks into one [128, 3*512] PSUM tile (3 banks) so a single
ACTIVATE(Exp) covers N=1536, amortizing the ~310-cycle ScalarE
per-instruction overhead -- exp on ScalarE (1 elem/cycle/lane @1.2GHz)
is the roofline engine at ~70us/core. PV accumulates [65, sw] in PSUM
over all 16 key-blocks; the epilogue is one DVE copy PSUM->SBUF and a
DMA (gpsimd queue, so it never blocks input prefetch) of the raw
[65, SP] d-major result. The host divides by the denominator row and
transposes/scatters back. Emission is software-pipelined (PE stream
[... QK(g), PV(g-2), QK(g+1), PV(g-1) ...]) so exp(g) hides under PE
work; measured steady state runs at ~93% ScalarE occupancy.
"""

import os
import sys

for _p in (
    "/root/.axon_site",
    "/root/.axon_site/_ro/trn_rl_repo",
    "/root/.axon_site/_ro/pypackages",
    "/opt/trn_rl_repo",
):
    if os.path.isdir(_p) and _p not in sys.path:
        sys.path.append(_p)

from concourse.bass_utils import run_bass_kernel_spmd

import numpy as np

import concourse.bacc as bacc
import concourse.tile as tile
import concourse.mybir as mybir

F32 = mybir.dt.float32
BF16 = mybir.dt.bfloat16

B, H = 2, 16
S, D = 2048, 64
N_CORES = 8
PAIRS_PER_CORE = (B * H) // N_CORES  # 4
CH = 512  # max chunk width (psum bank = 512 f32)


def _chunks(total, size):
    out, s0 = [], 0
    while s0 < total:
        w = min(size, total - s0)
        out.append((s0, w))
        s0 += w
    return out


def _groups(sw, nt):
    """Split nt key-blocks into ACT groups. Full chunks use 3-block groups
    (N=1536 exp per ACTIVATE, 3 psum banks); narrow chunks pack up to
    8 blocks while each matmul output stays inside one 2KB psum bank."""
    if sw == CH:
        sizes = []
        left = nt
        while left > 0:
            g = min(3, left)
            sizes.append(g)
            left -= g
    else:
        # keep every [128, sw] f32 matmul output within one bank
        per = max(1, min(nt, 2048 // (sw * 4)))
        sizes = []
        left = nt
        while left > 0:
            g = min(per, left)
            sizes.append(g)
            left -= g
    out, t0 = [], 0
    for g in sizes:
        out.append((t0, g))
        t0 += g
    return out


def build_attention_nc(NP=4, SP=1047, S_=2048, D_=64, row_tile=False, repeat=1):
    assert D_ == 64
    NT = S_ // 128

    nc = bacc.Bacc("TRN2", target_bir_lowering=False, debug=False)

    qt = nc.dram_tensor("qt", [NP, D_, SP], BF16, kind="ExternalInput")
    kt = nc.dram_tensor("kt", [NP, D_, S_], BF16, kind="ExternalInput")
    # v carries a host-appended ones column (softmax denominator trick)
    v = nc.dram_tensor("v", [NP, S_, D_ + 1], BF16, kind="ExternalInput")
    o = nc.dram_tensor("o", [NP, D_ + 1, SP], F32, kind="ExternalOutput")

    dma = nc.sync        # input prefetch on the HWDGE sync queue
    dma_out = nc.gpsimd  # outputs on the idle gpsimd queue so a pending
                         # output DMA never blocks input prefetch

    chunks = _chunks(SP, CH)

    with tile.TileContext(nc) as tc:
        with (
            tc.tile_pool(name="kt", bufs=3) as kt_pool,
            tc.tile_pool(name="v", bufs=3) as v_pool,
            tc.tile_pool(name="qt", bufs=3) as qt_pool,
            tc.tile_pool(name="exp", bufs=3) as exp_pool,
            tc.tile_pool(name="osb", bufs=2) as osb_pool,
            tc.tile_pool(name="qkps", bufs=2, space="PSUM") as qk_psum,
            tc.tile_pool(name="pvps", bufs=2, space="PSUM") as pv_psum,
        ):
            ctxs = {}

            def pair_prologue(p):
                nh = 2 if row_tile else 1
                # qT first: the pair's first QK matmul gates on it
                qt_sb = qt_pool.tile([128, SP], BF16, tag="qt")
                for h in range(nh):
                    dma.dma_start(qt_sb[h * 64 : h * 64 + 64, :], qt.ap()[p])
                kt_sb = kt_pool.tile([128, S_], BF16, tag="kt")
                for h in range(nh):
                    for c0 in range(0, S_, S_ // 2):
                        dma.dma_start(
                            kt_sb[h * 64 : h * 64 + 64, c0 : c0 + S_ // 2],
                            kt.ap()[p][:, c0 : c0 + S_ // 2],
                        )
                v_sb = v_pool.tile([128, NT, D_ + 1], BF16, tag="v")
                v_src = v.ap()[p].rearrange("(t p) d -> p t d", p=128)
                for t0 in range(0, NT, NT // 2):
                    dma.dma_start(
                        v_sb[:, t0 : t0 + NT // 2, :],
                        v_src[:, t0 : t0 + NT // 2, :],
                    )
                ctxs[p] = dict(kt=kt_sb, v=v_sb, qt=qt_sb)

            def emit_qk_group(p, s0, sw, t0g, glen):
                """QK matmuls for key-blocks t0g..t0g+glen-1, row-tiled by
                parity (half 0 / half 1 run concurrently on the PE)."""
                cx = ctxs[p]
                kt_sb, qt_sb = cx["kt"], cx["qt"]
                qk_t = qk_psum.tile([128, 3 * CH], F32, tag="qk")
                for j in range(glen):
                    t = t0g + j
                    h = (t % 2) if row_tile else 0
                    nc.tensor.matmul(
                        qk_t[:, j * sw : (j + 1) * sw],
                        kt_sb[h * 64 : h * 64 + 64, t * 128 : (t + 1) * 128],
                        qt_sb[h * 64 : h * 64 + 64, s0 : s0 + sw],
                        start=True,
                        stop=True,
                        tile_position=(h * 64, 0) if row_tile else None,
                    )
                return qk_t

            def emit_exp(p, sw, glen, qk_t):
                exp_t = exp_pool.tile([128, 3 * CH], BF16, tag="exp")
                nc.scalar.activation(
                    exp_t[:, 0 : glen * sw],
                    qk_t[:, 0 : glen * sw],
                    mybir.ActivationFunctionType.Exp,
                    scale=0.125,
                )
                return exp_t

            def make_pv(p, sw, t0g, glen, exp_t, pv_t):
                def emit():
                    v_sb = ctxs[p]["v"]
                    for j in range(glen):
                        t = t0g + j
                        nc.tensor.matmul(
                            pv_t[:, 0:sw],
                            v_sb[:, t, :],
                            exp_t[:, j * sw : (j + 1) * sw],
                            start=(t == 0),
                            stop=(t == NT - 1),
                            skip_group_check=True,
                        )

                return emit

            def make_epilogue(p, s0, sw, pv_t):
                def emit():
                    o_sb = osb_pool.tile([D_ + 1, CH], F32, tag="osb")
                    nc.vector.tensor_copy(o_sb[:, 0:sw], pv_t[:, 0:sw])
                    dma_out.dma_start(o.ap()[p][:, s0 : s0 + sw], o_sb[:, 0:sw])

                return emit

            # flat software-pipelined emission: PE stream is
            # [... QK(g), PV(g-2), QK(g+1), PV(g-1) ...] so the scalar
            # engine's exp(g) hides under PE work and never gates QK.
            def emit_body():
                gstep = [0]
                pvq = []
                epiq = []

                def tick():
                    gstep[0] += 1
                    while epiq and epiq[0][0] <= gstep[0]:
                        epiq.pop(0)[1]()
                    if len(pvq) >= 2:
                        pvq.pop(0)()

                for p in range(NP):
                    pair_prologue(p)
                    for s0, sw in chunks:
                        pv_t = pv_psum.tile([D_ + 1, CH], F32, tag="pv")
                        for t0g, glen in _groups(sw, NT):
                            qk_t = emit_qk_group(p, s0, sw, t0g, glen)
                            exp_t = emit_exp(p, sw, glen, qk_t)
                            tick()
                            pvq.append(make_pv(p, sw, t0g, glen, exp_t, pv_t))
                        epiq.append((gstep[0] + 3, make_epilogue(p, s0, sw, pv_t)))
                while pvq:
                    pvq.pop(0)()
                for _, fn in epiq:
                    fn()

            if repeat == 1:
                emit_body()
            else:
                with tc.For_i(0, repeat, 1):
                    emit_body()

    nc.compile()
    return nc


_NC_CACHE = {}
last_results = None


def _install_profile_hook():
    """Wire up the axon NTFF profiling hook if the image's antenv lacks it."""
    import types

    try:
        import antenv.axon_hooks  # noqa: F401

        return
    except ImportError:
        pass
    try:
        from trn_agent_boot.trn_boot import _ntff_profile_via_ctypes

        hook = _ntff_profile_via_ctypes("/opt/axon/libaxon_pjrt.so")
    except Exception:
        hook = None
    mod = types.ModuleType("antenv.axon_hooks")
    mod._hook = hook
    mod.get_axon_ntff_profile_hook = lambda: mod._hook
    mod.set_axon_ntff_profile_hook = lambda h: setattr(mod, "_hook", h)
    sys.modules["antenv.axon_hooks"] = mod
    import antenv

    antenv.axon_hooks = mod
    import concourse.bass_utils as _bu

    _bu.upload_artifacts = lambda tmpdir: "local://" + tmpdir


def prep_inputs(query, key, value, mask):
    """Compact/transpose/cast the full inputs into per-core shard maps."""
    import ml_dtypes

    query = np.asarray(query)
    key = np.asarray(key)
    value = np.asarray(value)
    mask = np.asarray(mask)

    idx = [np.nonzero(mask[b] != 0)[0] for b in range(B)]
    cnt = [len(ix) for ix in idx]
    # one zero-padded qT column per batch supplies colmean(V) for masked rows;
    # keep SP even so bf16 DMA rows stay 4-byte aligned
    SP = max(cnt) + (1 if min(cnt) < S else 0)
    SP += SP % 2

    in_maps = []
    for c in range(N_CORES):
        qs = np.zeros((PAIRS_PER_CORE, D, SP), dtype=ml_dtypes.bfloat16)
        ks = np.empty((PAIRS_PER_CORE, D, S), dtype=ml_dtypes.bfloat16)
        vs = np.empty((PAIRS_PER_CORE, S, D + 1), dtype=ml_dtypes.bfloat16)
        for i in range(PAIRS_PER_CORE):
            pair = c * PAIRS_PER_CORE + i
            b, h = pair // H, pair % H
            qs[i, :, : cnt[b]] = query[b, h, idx[b]].T
            ks[i] = key[b, h]
            vs[i, :, :D] = value[b, h]
            vs[i, :, D] = 1.0
        in_maps.append({"qt": qs, "kt": ks, "v": vs})
    return in_maps, idx, cnt, SP


def kernel(query, key, value, mask):
    """Full-input attention; shards over 8 NeuronCores internally."""
    global last_results
    in_maps, idx, cnt, SP = prep_inputs(query, key, value, mask)
    mask = np.asarray(mask)

    nc = _NC_CACHE.get(SP)
    if nc is None:
        nc = _NC_CACHE[SP] = build_attention_nc(
            NP=PAIRS_PER_CORE,
            SP=SP,
            row_tile=os.environ.get("KERNEL_ROW_TILE", "0") == "1",
        )

    trace = os.environ.get("KERNEL_PROFILE", "") == "1"
    if trace:
        _install_profile_hook()
        try:
            import jax

            jax.device_put(
                np.zeros((4,), np.float32), jax.devices()[0]
            ).block_until_ready()
        except Exception as e:
            print(f"profile warmup failed ({e}); disabling trace", file=sys.stderr)
            trace = False
    res = run_bass_kernel_spmd(nc, in_maps, core_ids=list(range(N_CORES)), trace=trace)
    last_results = res

    out = np.empty((B, H, S, D), dtype=np.float32)
    for c in range(N_CORES):
        oc = res.results[c]["o"]  # [NP, D+1, SP] f32 (raw PV + denominator row)
        for i in range(PAIRS_PER_CORE):
            pair = c * PAIRS_PER_CORE + i
            b, h = pair // H, pair % H
            on = oc[i, :D, :] / oc[i, D : D + 1, :]
            out[b, h, idx[b]] = on[:, : cnt[b]].T
            if cnt[b] < S:
                out[b, h, np.nonzero(mask[b] == 0)[0]] = on[:, cnt[b]]
    return out
